# revision 1
# baseline (speedup 1.0000x reference)
"""Trainium2 Bass kernel for nn_LoraQKV (MLA-style LoRA QKV + causal SDPA + out-proj).

Strategy (8 NeuronCores, tensor-parallel over heads):
  - Each core owns 4 of the 32 heads: Wqb/Wkvb row-sharded by head, Wo
    column-sharded; per-core partial outputs summed on the host.
  - The LoRA down-projections (hidden @ Wqa.T / Wkva.T) + RMSNorms are
    replicated on every core (they feed all heads).
  - All matmuls run in bf16 (weights/activations pre-cast on host), f32
    PSUM accumulation. RMSNorm weight vectors are folded into Wqb/Wkvb
    on the host (they multiply the same axis that is contracted).
  - Scores are computed causally (lower-triangle blocks only); softmax
    skips max-subtraction (scores are bounded ~N(0, 0.4^2) by
    construction) and folds the row-sum into the Exp activation's
    accum_out. probs rows are scaled by 1/denom before transposition so
    no partition-broadcast multiply is ever needed.
"""

import math
import os
import sys
import types

import numpy as np
import ml_dtypes

import concourse.bass as bass
import concourse.mybir as mybir
import concourse.tile as tile
from concourse import masks

BF16 = mybir.dt.bfloat16
F32 = mybir.dt.float32

H = 32
HD = 64
MD = 64
S = 2048
D = 2048
QR = 1024
KVR = 896
EPS = 1e-6
SCALING = (HD + MD) ** -0.5
NCORES = 8
HPC = H // NCORES          # heads per core = 4
ST = S // 128              # 16 s-tiles
DC = D // 128              # 16 d-chunks
QRC = QR // 128            # 8
KVC = KVR // 128           # 7

AF = mybir.ActivationFunctionType
AX = mybir.AxisListType


def _split_sync_waits(nc, maxw=1):
    """This container's walrus build rejects instructions with more than one
    semaphore wait (core_v3 setupSyncWait). Move excess waits onto NoOps
    emitted just before the instruction on the same (in-order) engine."""
    for fn in nc.m.functions:
        for bb in fn.blocks:
            insts = bb.instructions
            new = []
            for inst in insts:
                si = getattr(inst, "sync_info", None)
                waits = list(si.on_wait) if si is not None and si.on_wait else []
                if len(waits) > maxw and getattr(inst, "engine", None) is not None:
                    excess, keep = waits[:-maxw], waits[-maxw:]
                    for j in range(0, len(excess), maxw):
                        new.append(
                            mybir.InstNoOp(
                                name=f"{inst.name}.w{j}",
                                engine=inst.engine,
                                sync_info=mybir.SyncInfo(
                                    on_wait=excess[j : j + maxw], on_update=[]
                                ),
                                bass_nofuse=True,
                            )
                        )
                    inst.sync_info = mybir.SyncInfo(
                        on_wait=keep, on_update=list(si.on_update)
                    )
                new.append(inst)
            bb.instructions = new


def build_module():
    nc = bass.Bass(target_bir_lowering=False)

    # ---- DRAM I/O (per-core shards supplied via in_maps) ----
    hTb = nc.dram_tensor("hTb", [ST, 128, DC, 128], BF16, kind="ExternalInput")
    wqaT = nc.dram_tensor("wqaT", [DC, 128, QR], BF16, kind="ExternalInput")
    wkvaT = nc.dram_tensor("wkvaT", [DC, 128, KVR + MD], BF16, kind="ExternalInput")
    wqbT = nc.dram_tensor("wqbT", [QRC, 128, HPC * 128], BF16, kind="ExternalInput")
    wkT = nc.dram_tensor("wkT", [KVC, 128, HPC * 64], BF16, kind="ExternalInput")
    wvT = nc.dram_tensor("wvT", [KVC, 128, HPC * 64], BF16, kind="ExternalInput")
    woT = nc.dram_tensor("woT", [2, 128, D], BF16, kind="ExternalInput")
    cosT_d = nc.dram_tensor("cosT", [MD, S], F32, kind="ExternalInput")
    sinT_d = nc.dram_tensor("sinT", [MD, S], F32, kind="ExternalInput")
    maskneg_d = nc.dram_tensor("maskneg", [128, 128], F32, kind="ExternalInput")
    out_d = nc.dram_tensor("out", [S, D], F32, kind="ExternalOutput")

    with tile.TileContext(nc, pool_alloc_mode="queue") as tc:
        import contextlib

        with contextlib.ExitStack() as ctx:
            # ---------- long-lived pools ----------
            pconst = ctx.enter_context(tc.tile_pool(name="const", bufs=1))
            ident_bf = pconst.tile([128, 128], BF16)
            ident_f = pconst.tile([128, 128], F32)
            masks.make_identity(nc, ident_bf[:])
            masks.make_identity(nc, ident_f[:])
            maskneg = pconst.tile([128, 128], F32)
            nc.sync.dma_start(maskneg[:], maskneg_d[:])
            cosT = pconst.tile([MD, S], F32)
            sinT = pconst.tile([MD, S], F32)
            nc.sync.dma_start(cosT[:], cosT_d[:])
            nc.sync.dma_start(sinT[:], sinT_d[:])

            p_qnT = ctx.enter_context(tc.tile_pool(name="qnT", bufs=1))
            qnT = p_qnT.tile([128, QRC, S], BF16)        # cq_norm^T
            p_knT = ctx.enter_context(tc.tile_pool(name="knT", bufs=1))
            knT = p_knT.tile([128, KVC, S], BF16)        # ckv_norm^T
            p_krT = ctx.enter_context(tc.tile_pool(name="krT", bufs=1))
            krT = p_krT.tile([64, S], BF16)              # roped shared k_rope^T

            p_qkv = ctx.enter_context(tc.tile_pool(name="qkv", bufs=1))
            qT = p_qkv.tile([128, HPC, S], BF16)         # per-head q^T (roped)
            kTc = p_qkv.tile([128, HPC, S], BF16)        # per-head [k_nope; k_rope]^T
            V_sb = p_qkv.tile([128, ST, HPC * 64], BF16)  # V per s-tile

            p_aoT = ctx.enter_context(tc.tile_pool(name="aoT", bufs=1))
            aoT = p_aoT.tile([128, 2, S], BF16)          # attn-out^T (scaled)

            # ---------- Phase 1: down-projections + RMSNorm + transposes ----------
            def down_phase(w_sb, wdram, ncols, psum_tag, do_norm_cols, invn, kv_rope):
                # loads weights w_sb <- wdram, then per s-tile:
                #   accum psum = hT_blk^T-weighted matmuls, norm, transpose.
                for d in range(DC):
                    nc.sync.dma_start(w_sb[:, d, :], wdram[d])

                with tc.tile_pool(name=f"ph1_{psum_tag}", bufs=1) as sb1, \
                     tc.tile_pool(name=f"pp1_{psum_tag}", bufs=1, space="PSUM") as pp1, \
                     tc.tile_pool(name=f"pt1_{psum_tag}", bufs=2, space="PSUM") as pt1:
                    for t in range(ST):
                        hbig = sb1.tile([128, DC * 128], BF16, tag="hbig", bufs=3)
                        nc.sync.dma_start(hbig[:], hTb[t])
                        pacc = pp1.tile([128, ncols], F32, tag="pacc")
                        for d in range(DC):
                            for n0 in range(0, ncols, 512):
                                w = min(512, ncols - n0)
                                nc.tensor.matmul(
                                    pacc[:, n0 : n0 + w],
                                    hbig[:, d * 128 : (d + 1) * 128],
                                    w_sb[:, d, n0 : n0 + w],
                                    start=(d == 0),
                                    stop=(d == DC - 1),
                                )
                        # RMSNorm over the first do_norm_cols columns
                        sq = sb1.tile([128, do_norm_cols], F32, tag="sq", bufs=2)
                        var = sb1.tile([128, 1], F32, tag="var", bufs=2)
                        nc.scalar.activation(
                            sq[:], pacc[:, 0:do_norm_cols], AF.Square,
                            scale=float(invn), accum_out=var[:],
                        )
                        nc.vector.tensor_scalar_add(var[:], var[:], EPS)
                        rstd = sb1.tile([128, 1], F32, tag="rstd", bufs=2)
                        nc.vector.reciprocal(rstd[:], var[:])
                        nc.scalar.sqrt(rstd[:], rstd[:])
                        nrm = sb1.tile([128, do_norm_cols], BF16, tag="nrm", bufs=2)
                        nc.vector.tensor_scalar_mul(nrm[:], pacc[:, 0:do_norm_cols], rstd[:])
                        # transpose normed block-columns into the ^T layout
                        nblk = do_norm_cols // 128
                        destT = qnT if psum_tag == "q" else knT
                        for g0 in range(0, nblk, 4):
                            gn = min(4, nblk - g0)
                            ptr = pt1.tile([128, 512], BF16, tag="tr")
                            for j in range(gn):
                                nc.tensor.transpose(
                                    ptr[:, j * 128 : (j + 1) * 128],
                                    nrm[:, (g0 + j) * 128 : (g0 + j + 1) * 128],
                                    ident_bf[:],
                                )
                            src = ptr[:, 0 : gn * 128].rearrange(
                                "p (c k) -> p c k", k=128
                            )
                            nc.vector.tensor_copy(
                                destT[:, g0 : g0 + gn, t * 128 : (t + 1) * 128], src
                            )
                        if kv_rope:
                            # raw k_rope cols 896:960 -> f32 SBUF -> transpose ->
                            # rope in ^T layout -> krT (bf16)
                            krs = sb1.tile([128, MD], F32, tag="krs", bufs=2)
                            nc.scalar.copy(krs[:], pacc[:, KVR : KVR + MD])
                            ptf = pt1.tile([64, 128], F32, tag="trf", bufs=2)
                            nc.tensor.transpose(ptf[:], krs[:], ident_f[:])
                            cs = slice(t * 128, (t + 1) * 128)
                            tmp1 = sb1.tile([32, 128], F32, tag="rtmp1", bufs=2)
                            tmp2 = sb1.tile([32, 128], F32, tag="rtmp2", bufs=2)
                            nc.vector.tensor_mul(tmp1[:], ptf[0:32, :], cosT[0:32, cs])
                            nc.vector.tensor_mul(tmp2[:], ptf[32:64, :], sinT[0:32, cs])
                            nc.vector.tensor_sub(krT[0:32, cs], tmp1[:], tmp2[:])
                            tmp3 = sb1.tile([32, 128], F32, tag="rtmp3", bufs=2)
                            tmp4 = sb1.tile([32, 128], F32, tag="rtmp4", bufs=2)
                            nc.vector.tensor_mul(tmp3[:], ptf[32:64, :], cosT[32:64, cs])
                            nc.vector.tensor_mul(tmp4[:], ptf[0:32, :], sinT[32:64, cs])
                            nc.vector.tensor_add(krT[32:64, cs], tmp3[:], tmp4[:])

            with tc.tile_pool(name="wqa_pool", bufs=1) as pw:
                wqa_sb = pw.tile([128, DC, QR], BF16)
                down_phase(wqa_sb, wqaT, QR, "q", QR, 1.0 / math.sqrt(QR), False)
            with tc.tile_pool(name="wkva_pool", bufs=1) as pw:
                wkva_sb = pw.tile([128, DC, KVR + MD], BF16)
                down_phase(wkva_sb, wkvaT, KVR + MD, "kv", KVR,
                           1.0 / math.sqrt(KVR), True)

            # ---------- Phase 2: up-projections (+ q rope) ----------
            with tc.tile_pool(name="p2w", bufs=1) as p2w, \
                 tc.tile_pool(name="p2s", bufs=2) as p2s, \
                 tc.tile_pool(name="pp2", bufs=4, space="PSUM") as pp2, \
                 tc.tile_pool(name="pp2v", bufs=2, space="PSUM") as pp2v:
                wqb_sb = p2w.tile([128, QRC, HPC * 128], BF16)
                for r in range(QRC):
                    nc.sync.dma_start(wqb_sb[:, r, :], wqbT[r])
                wk_sb = p2w.tile([128, KVC, HPC * 64], BF16)
                wv_sb = p2w.tile([128, KVC, HPC * 64], BF16)
                for r in range(KVC):
                    nc.sync.dma_start(wk_sb[:, r, :], wkT[r])
                    nc.sync.dma_start(wv_sb[:, r, :], wvT[r])

                for sc in range(4):  # 512-wide s chunks
                    cols = slice(sc * 512, (sc + 1) * 512)
                    for ht in range(HPC):
                        pq = pp2.tile([128, 512], F32, tag="mm")
                        for r in range(QRC):
                            nc.tensor.matmul(
                                pq[:], wqb_sb[:, r, ht * 128 : (ht + 1) * 128],
                                qnT[:, r, cols], start=(r == 0), stop=(r == QRC - 1),
                            )
                        # nope rows straight copy; rope rows get rotary
                        nc.scalar.copy(qT[0:64, ht, cols], pq[0:64, :])
                        a1 = p2s.tile([32, 512], F32, tag="qa1")
                        a2 = p2s.tile([32, 512], F32, tag="qa2")
                        nc.vector.tensor_mul(a1[:], pq[64:96, :], cosT[0:32, cols])
                        nc.vector.tensor_mul(a2[:], pq[96:128, :], sinT[0:32, cols])
                        nc.vector.tensor_sub(qT[64:96, ht, cols], a1[:], a2[:])
                        a3 = p2s.tile([32, 512], F32, tag="qa3")
                        a4 = p2s.tile([32, 512], F32, tag="qa4")
                        nc.vector.tensor_mul(a3[:], pq[96:128, :], cosT[32:64, cols])
                        nc.vector.tensor_mul(a4[:], pq[64:96, :], sinT[32:64, cols])
                        nc.vector.tensor_add(qT[96:128, ht, cols], a3[:], a4[:])
                    for kt in range(2):
                        pk = pp2.tile([128, 512], F32, tag="mm")
                        for r in range(KVC):
                            nc.tensor.matmul(
                                pk[:], wk_sb[:, r, kt * 128 : (kt + 1) * 128],
                                knT[:, r, cols], start=(r == 0), stop=(r == KVC - 1),
                            )
                        nc.scalar.copy(kTc[0:64, 2 * kt, cols], pk[0:64, :])
                        nc.scalar.copy(kTc[0:64, 2 * kt + 1, cols], pk[64:128, :])
                    for h in range(HPC):
                        nc.vector.tensor_copy(kTc[64:128, h, cols], krT[:, cols])
                for st in range(ST):
                    pv = pp2v.tile([128, HPC * 64], F32, tag="mmv")
                    for r in range(KVC):
                        nc.tensor.matmul(
                            pv[:], knT[:, r, st * 128 : (st + 1) * 128], wv_sb[:, r, :],
                            start=(r == 0), stop=(r == KVC - 1),
                        )
                    nc.vector.tensor_copy(V_sb[:, st, :], pv[:])

            # ---------- Phase 3: causal attention per head ----------
            with tc.tile_pool(name="p3s", bufs=2) as p3s, \
                 tc.tile_pool(name="p3pt", bufs=17) as p3pt, \
                 tc.tile_pool(name="pp3s", bufs=2, space="PSUM") as pp3s, \
                 tc.tile_pool(name="pp3t", bufs=3, space="PSUM") as pp3t, \
                 tc.tile_pool(name="pp3o", bufs=2, space="PSUM") as pp3o:
                for h in range(HPC):
                    for qc in range(4):  # 512-wide q chunks
                        nkc = 4 * (qc + 1)  # k-chunks (128-wide) this q-chunk sees
                        pT = [
                            p3pt.tile([128, 512], BF16, tag="pT", name=f"pT{h}_{qc}_{k}")
                            for k in range(nkc)
                        ]
                        for tl in range(4):
                            t = qc * 4 + tl
                            nk = (t + 1) * 128
                            nch = (nk + 511) // 512
                            probs = p3s.tile([128, S], BF16, tag="probs")
                            dparts = p3s.tile([128, 4], F32, tag="dparts")
                            for ci, k0 in enumerate(range(0, nk, 512)):
                                w = min(512, nk - k0)
                                ps = pp3s.tile([128, 512], F32, tag="ps")
                                nc.tensor.matmul(
                                    ps[:, 0:w],
                                    qT[:, h, t * 128 : (t + 1) * 128],
                                    kTc[:, h, k0 : k0 + w],
                                )
                                if k0 + w == nk:  # diagonal block: causal mask
                                    nc.vector.tensor_add(
                                        ps[:, w - 128 : w], ps[:, w - 128 : w],
                                        maskneg[:],
                                    )
                                nc.scalar.activation(
                                    probs[:, k0 : k0 + w], ps[:, 0:w], AF.Exp,
                                    scale=float(SCALING),
                                    accum_out=dparts[:, ci : ci + 1],
                                )
                            den = p3s.tile([128, 1], F32, tag="den")
                            nc.vector.reduce_sum(den[:], dparts[:, 0:nch], axis=AX.X)
                            rcp = p3s.tile([128, 1], F32, tag="rcp")
                            nc.vector.reciprocal(rcp[:], den[:])
                            nc.vector.tensor_scalar_mul(
                                probs[:, 0:nk], probs[:, 0:nk], rcp[:]
                            )
                            for kb in range(t + 1):
                                ptr = pp3t.tile([128, 128], BF16, tag="tr3")
                                nc.tensor.transpose(
                                    ptr[:], probs[:, kb * 128 : (kb + 1) * 128],
                                    ident_bf[:],
                                )
                                nc.vector.tensor_copy(
                                    pT[kb][:, tl * 128 : (tl + 1) * 128], ptr[:]
                                )
                        po = pp3o.tile([64, 512], F32, tag="po")
                        for kc in range(nkc):
                            i = max(0, kc - 4 * qc)  # causal-zero cols < i*128
                            nc.tensor.matmul(
                                po[:, i * 128 : 512],
                                V_sb[:, kc, h * 64 : (h + 1) * 64],
                                pT[kc][:, i * 128 : 512],
                                start=(kc == 0),
                                stop=(kc == nkc - 1),
                                skip_group_check=True,
                            )
                        nc.vector.tensor_copy(
                            aoT[(h % 2) * 64 : (h % 2) * 64 + 64, h // 2,
                                qc * 512 : (qc + 1) * 512],
                            po[:],
                        )

            # ---------- Phase 4: partial out-projection ----------
            with tc.tile_pool(name="p4w", bufs=1) as p4w, \
                 tc.tile_pool(name="p4s", bufs=3) as p4s, \
                 tc.tile_pool(name="pp4", bufs=4, space="PSUM") as pp4:
                wo_sb = p4w.tile([128, 2, D], BF16)
                nc.sync.dma_start(wo_sb[:, 0, :], woT[0])
                nc.sync.dma_start(wo_sb[:, 1, :], woT[1])
                for t in range(ST):
                    ysb = p4s.tile([128, D], F32, tag="ysb")
                    for n in range(4):
                        py = pp4.tile([128, 512], F32, tag="py")
                        for cc in range(2):
                            nc.tensor.matmul(
                                py[:],
                                aoT[:, cc, t * 128 : (t + 1) * 128],
                                wo_sb[:, cc, n * 512 : (n + 1) * 512],
                                start=(cc == 0),
                                stop=(cc == 1),
                            )
                        nc.scalar.copy(ysb[:, n * 512 : (n + 1) * 512], py[:])
                    nc.sync.dma_start(out_d[t * 128 : (t + 1) * 128, :], ysb[:])

    _split_sync_waits(nc)
    return nc


def prep_in_maps(hidden_states, cos, sin, Wqa, q_ln_w, Wqb, Wkva, kv_ln_w, Wkvb, Wo):
    bf = ml_dtypes.bfloat16
    f32 = np.float32
    h = np.asarray(hidden_states, f32).reshape(S, D)
    # hTb[t, p, c, j] = h[t*128 + j, c*128 + p] : per-(t) contiguous lhsT blocks
    hTb = np.ascontiguousarray(
        h.reshape(ST, 128, DC, 128).transpose(0, 3, 2, 1)
    ).astype(bf)
    wqaT = np.ascontiguousarray(np.asarray(Wqa, f32).T).astype(bf).reshape(DC, 128, QR)
    wkvaT = (
        np.ascontiguousarray(np.asarray(Wkva, f32).T).astype(bf).reshape(DC, 128, KVR + MD)
    )
    wqb_eff = np.asarray(Wqb, f32) * np.asarray(q_ln_w, f32)[None, :]
    wkvb_eff = np.asarray(Wkvb, f32) * np.asarray(kv_ln_w, f32)[None, :]
    cosT = np.ascontiguousarray(np.asarray(cos, f32).T)
    sinT = np.ascontiguousarray(np.asarray(sin, f32).T)
    maskneg = np.where(
        np.arange(128)[None, :] > np.arange(128)[:, None], -1e9, 0.0
    ).astype(f32)

    shared = {
        "hTb": hTb, "wqaT": wqaT, "wkvaT": wkvaT,
        "cosT": cosT, "sinT": sinT, "maskneg": maskneg,
    }
    in_maps = []
    for c in range(NCORES):
        wqb_c = wqb_eff[c * HPC * 128 : (c + 1) * HPC * 128]        # [512, QR]
        wqbT_c = np.ascontiguousarray(wqb_c.T).astype(bf).reshape(QRC, 128, HPC * 128)
        wkvb_c = wkvb_eff[c * HPC * 128 : (c + 1) * HPC * 128].reshape(HPC, 2, 64, KVR)
        wk_c = np.ascontiguousarray(
            wkvb_c[:, 0].reshape(HPC * 64, KVR).T
        ).astype(bf).reshape(KVC, 128, HPC * 64)
        wv_c = np.ascontiguousarray(
            wkvb_c[:, 1].reshape(HPC * 64, KVR).T
        ).astype(bf).reshape(KVC, 128, HPC * 64)
        wo_c = np.ascontiguousarray(
            np.asarray(Wo, f32)[:, c * HPC * 64 : (c + 1) * HPC * 64].T
        ).astype(bf).reshape(2, 128, D)
        in_maps.append(
            dict(shared, wqbT=wqbT_c, wkT=wk_c, wvT=wv_c, woT=wo_c)
        )
    return in_maps


_NC_CACHE = []


def kernel(**inputs):
    from concourse.bass_utils import run_bass_kernel_spmd

    in_maps = prep_in_maps(**inputs)
    if not _NC_CACHE:
        _NC_CACHE.append(build_module())
    nc = _NC_CACHE[0]
    res = run_bass_kernel_spmd(nc, in_maps, core_ids=list(range(NCORES)))
    total = np.zeros((S, D), np.float32)
    for r in res.results:
        total += np.asarray(r["out"], np.float32)
    return total.reshape(1, S, D)
